# revision 1
# baseline (speedup 1.0000x reference)
"""Trainium2 Bass kernel for CrossKGAttention (bidirectional masked cross-attention
between two knowledge-graph embedding sets).

Math per direction (queries q_emb [Nq,256], kv kv_emb [Nk,256], mask A [Nq,Nk]):
  Q_i = q_emb @ Wq.T + bq            (head i slice, [Nq,64])
  Kbar = mean_i(kv_emb @ Wk.T + bk)  ([Nk,64])
  V_i  = kv_emb @ Wv.T + bv
  S_i  = Q_i @ Kbar.T * SCALE
  w    = softmax(S_i * A, axis=kv)
  out_i = w @ V_i ;  enhanced = q_emb + out @ Wo.T + bo

Key rewrite used on device: with E = (exp(S)-1) * A  (exactly 0 where A==0),
  unnorm_i = E_i^T-weighted V_i + sum_m V_i[m]     (ones column gives sum_m E)
  denom_i  = Nk + sum_m E_i
  out_i    = unnorm_i / denom_i + bv
All score/exp/PV work happens in the transposed [kv, query] layout so the PV
contraction runs at full PE efficiency with no large transposes; only the tiny
[65, nq] per-head results are transposed back via the PE identity trick.

Sharding: 8 cores; core c owns kg1 query rows [c*750,(c+1)*750) for direction
1->2 and kg2 query rows for 2->1. K/V sources + weights replicated. Queries are
padded 750->768 so every matmul chunk is 256 wide (fp32r full rate, PSUM-bank
aligned).
"""

import numpy as np
import ml_dtypes
from contextlib import ExitStack

import concourse.bass as bass
import concourse.tile as tile
from concourse import bacc, mybir
from concourse.bass_utils import run_bass_kernel_spmd

F32 = mybir.dt.float32
F32R = mybir.dt.float32r
BF16 = mybir.dt.bfloat16
NPBF16 = ml_dtypes.bfloat16

N = 6000          # entities per KG (both sides)
HID = 256
HEADS = 4
D = 64
SCALE = D ** -0.5
NCORES = 8
NQ = N // NCORES          # 750 queries per core per direction
NQP = 768                 # padded queries (3 chunks of 256)
NSZ = 256                 # n-chunk size
NCHUNK = NQP // NSZ       # 3
MBS = 128                 # m-block size
NMB = (N + MBS - 1) // MBS   # 47 (46 full + 112)


def _r32(ap):
    return ap.bitcast(F32R)


def _build_kernel(ctx: ExitStack, tc, ins, outs):
    nc = tc.nc
    (e1T, e2T, eq1T, eq2T, wqT, wkbT, wvT, woT,
     bq_h, bkb, bv2, bo2, a1T, a2T, id128) = ins
    o1T, o2T = outs

    ctx.enter_context(nc.allow_low_precision(reason="fp32r storage is fp32 bits"))
    consts = ctx.enter_context(tc.tile_pool(name="consts", bufs=1))
    perdir = ctx.enter_context(tc.tile_pool(name="perdir", bufs=1))
    small2 = ctx.enter_context(tc.tile_pool(name="small2", bufs=3))
    maskp = ctx.enter_context(tc.tile_pool(name="maskp", bufs=16))
    expp = ctx.enter_context(tc.tile_pool(name="expp", bufs=4))
    ep = ctx.enter_context(tc.tile_pool(name="ep", bufs=4))
    asm = ctx.enter_context(tc.tile_pool(name="asm", bufs=3))
    outp = ctx.enter_context(tc.tile_pool(name="outp", bufs=4))

    # ---- resident constants ----
    wq_sb = consts.tile([128, 2, HID], F32R)
    nc.sync.dma_start(out=wq_sb[:], in_=wqT.rearrange("(b p) h -> p b h", p=128))
    wv_sb = consts.tile([128, 2, HID], F32R)
    nc.sync.dma_start(out=wv_sb[:], in_=wvT.rearrange("(b p) h -> p b h", p=128))
    wo_sb = consts.tile([128, 2, HID], F32R)
    nc.sync.dma_start(out=wo_sb[:], in_=woT.rearrange("(b p) h -> p b h", p=128))
    wkb_sb = consts.tile([128, 2, D], F32R)
    nc.sync.dma_start(out=wkb_sb[:], in_=wkbT.rearrange("(b p) d -> p b d", p=128))
    id_sb = consts.tile([128, 128], F32)
    nc.sync.dma_start(out=id_sb[:], in_=id128[:, :])
    bq_sb = consts.tile([64, HEADS], F32)
    nc.sync.dma_start(out=bq_sb[:], in_=bq_h[:, :])
    bkb_sb = consts.tile([64, 1], F32)
    nc.sync.dma_start(out=bkb_sb[:], in_=bkb[:, :])
    bv_sb = consts.tile([128, 2], F32)
    nc.sync.dma_start(out=bv_sb[:], in_=bv2[:, :])
    bo_sb = consts.tile([128, 2], F32)
    nc.sync.dma_start(out=bo_sb[:], in_=bo2[:, :])

    for dirx in range(2):
        ekvT_d = e2T if dirx == 0 else e1T
        eqT_d = eq1T if dirx == 0 else eq2T
        maskT_d = a1T if dirx == 0 else a2T
        oT_d = o1T if dirx == 0 else o2T

        # ---- load embeddings ----
        ekv_sb = perdir.tile([128, 2, N], F32R, tag="ekv")
        nc.sync.dma_start(out=ekv_sb[:],
                          in_=ekvT_d.rearrange("(b p) m -> p b m", p=128))
        eq_sb = small2.tile([128, 2, NQP], F32R, tag="eq")
        nc.sync.dma_start(out=eq_sb[:],
                          in_=eqT_d.rearrange("(b p) m -> p b m", p=128))

        kb_sb = perdir.tile([64, N], F32R, tag="kb")
        q_sb = perdir.tile([64, HEADS, NQP], F32R, tag="q")
        vt_sb = perdir.tile([128, NMB, HEADS, D + 1], BF16, tag="vt")
        vsum_sb = small2.tile([64, HEADS], F32, tag="vsum")
        es_sb = small2.tile([128, 2], F32R, tag="es")

        with tc.tile_pool(name="projps", bufs=3, space="PSUM") as projps:
            # KbarT projection: [64, N] = wkbT.T @ ekvT  (fp32r)
            for chn in range(N // 500 + 1):
                c0 = chn * 500
                cw = min(500, N - c0)
                if cw <= 0:
                    break
                ps = projps.tile([128, 512], F32, tag="proj")
                for kb in range(2):
                    nc.tensor.matmul(ps[0:64, 0:cw],
                                     _r32(wkb_sb[:, kb, :]),
                                     _r32(ekv_sb[:, kb, c0:c0 + cw]),
                                     start=(kb == 0), stop=(kb == 1))
                nc.vector.tensor_scalar_add(kb_sb[:, c0:c0 + cw],
                                            ps[0:64, 0:cw], bkb_sb[:, 0:1])

            # Q projection per head: [64, NQP]
            for h in range(HEADS):
                for chn in range(2):
                    c0 = chn * 384
                    ps = projps.tile([128, 512], F32, tag="proj")
                    for kb in range(2):
                        nc.tensor.matmul(
                            ps[0:64, 0:384],
                            _r32(wq_sb[:, kb, h * D:(h + 1) * D]),
                            _r32(eq_sb[:, kb, c0:c0 + 384]),
                            start=(kb == 0), stop=(kb == 1))
                    nc.vector.tensor_scalar_add(q_sb[:, h, c0:c0 + 384],
                                                ps[0:64, 0:384],
                                                bq_sb[:, h:h + 1])

            # V projection (natural layout) -> vt_sb bf16 with ones column
            nc.vector.memset(vt_sb[:, :, :, D:D + 1], 1.0)
            for mb in range(NMB):
                m0 = mb * MBS
                mw = min(MBS, N - m0)
                ps = projps.tile([128, 512], F32, tag="proj")
                for kb in range(2):
                    nc.tensor.matmul(ps[0:mw, 0:HID],
                                     _r32(ekv_sb[:, kb, m0:m0 + mw]),
                                     _r32(wv_sb[:, kb, :]),
                                     start=(kb == 0), stop=(kb == 1))
                src = ps[0:mw, 0:HID].rearrange("p (h d) -> p h d", h=HEADS)
                nc.vector.tensor_copy(vt_sb[0:mw, mb, :, 0:D], src)

            # Vsum per head: embsum (DVE free-reduce) then tiny matmuls
            for kb in range(2):
                nc.vector.tensor_reduce(es_sb[:, kb:kb + 1], ekv_sb[:, kb, :],
                                        axis=mybir.AxisListType.X,
                                        op=mybir.AluOpType.add)
            psv = projps.tile([128, 512], F32, tag="proj")
            for h in range(HEADS):
                for kb in range(2):
                    nc.tensor.matmul(psv[0:64, h:h + 1],
                                     wv_sb[:, kb, h * D:(h + 1) * D].bitcast(F32),
                                     es_sb[:, kb:kb + 1].bitcast(F32),
                                     start=(kb == 0), stop=(kb == 1))
            nc.vector.tensor_copy(vsum_sb[:, :], psv[0:64, 0:HEADS])

        oT_sb = perdir.tile([128, 2, NQP], F32R, tag="oT")

        with (tc.tile_pool(name="scrp", bufs=2, space="PSUM") as scrp,
              tc.tile_pool(name="pvp", bufs=1, space="PSUM") as pvp):
          asmps = scrp
          # ---- main loop: per n-chunk of 256 queries ----
          for nt in range(NCHUNK):
            n0 = nt * NSZ
            pv = pvp.tile([D + 1, HEADS, 512], F32, tag="pv")
            for mb in range(NMB):
                m0 = mb * MBS
                mw = min(MBS, N - m0)
                a_t = maskp.tile([128, NSZ], BF16, tag="mask")
                nc.sync.dma_start(out=a_t[0:mw, :],
                                  in_=maskT_d[m0:m0 + mw, n0:n0 + NSZ])
                scr = scrp.tile([128, HEADS, NSZ], F32, tag="scr")
                for h in range(HEADS):
                    nc.tensor.matmul(scr[0:mw, h, :],
                                     _r32(kb_sb[:, m0:m0 + mw]),
                                     _r32(q_sb[:, h, n0:n0 + NSZ]),
                                     start=True, stop=True)
                exp_t = expp.tile([128, HEADS, NSZ], BF16, tag="exp")
                nc.scalar.activation(out=exp_t[0:mw, :, :], in_=scr[0:mw, :, :],
                                     func=mybir.ActivationFunctionType.Exp)
                e_t = ep.tile([128, HEADS, NSZ], BF16, tag="e")
                a_ap = a_t[0:mw, :]
                a_brd = bass.AP(a_ap.tensor, a_ap.offset,
                                [a_ap.ap[0], [0, HEADS], a_ap.ap[1]])
                nc.vector.scalar_tensor_tensor(
                    out=e_t[0:mw, :, :], in0=exp_t[0:mw, :, :], scalar=1.0,
                    in1=a_brd,
                    op0=mybir.AluOpType.subtract, op1=mybir.AluOpType.mult)
                for h in range(HEADS):
                    nc.tensor.matmul(pv[:, h, 0:NSZ],
                                     vt_sb[0:mw, mb, h, :],
                                     e_t[0:mw, h, :],
                                     start=(mb == 0), stop=(mb == NMB - 1))

            # ---- assembly for this n-chunk ----
            p_sb = asm.tile([D + 1, HEADS, NSZ], F32, tag="p")
            for h in range(HEADS):
                nc.vector.tensor_scalar_add(p_sb[0:D, h, :], pv[0:D, h, 0:NSZ],
                                            vsum_sb[:, h:h + 1])
            nc.vector.tensor_scalar_add(p_sb[D:D + 1, :, :],
                                        pv[D:D + 1, :, 0:NSZ], float(N))
            for c in range(2):
                q0 = c * 128
                on_t = asm.tile([128, HEADS, D], F32, tag="onat")
                for h in range(HEADS):
                    trt = asmps.tile([128, HEADS, NSZ], F32, tag="scr")
                    tr = trt[:].rearrange("p a b -> p (a b)")
                    nc.tensor.transpose(tr[0:128, 0:D + 1],
                                        p_sb[0:D + 1, h, q0:q0 + 128],
                                        id_sb[0:D + 1, 0:D + 1])
                    dv = asm.tile([128, 1], F32, tag="dv")
                    nc.vector.reciprocal(dv[:, :], tr[0:128, D:D + 1])
                    nc.vector.tensor_scalar_mul(on_t[:, h, :], tr[0:128, 0:D],
                                                dv[:, 0:1])
                for hb in range(2):
                    trbt = asmps.tile([128, HEADS, NSZ], F32, tag="scr")
                    trb = trbt[:].rearrange("p a b -> p (a b)")
                    srcv = on_t[:].rearrange("p h d -> p (h d)")
                    nc.tensor.transpose(trb[0:128, 0:128],
                                        srcv[:, hb * 128:(hb + 1) * 128],
                                        id_sb[:, :])
                    nc.vector.tensor_scalar_add(
                        oT_sb[:, hb, n0 + q0:n0 + q0 + 128],
                        trb[0:128, 0:128], bv_sb[:, hb:hb + 1])

          # ---- Wo projection + residual ----
          for hb in range(2):
              for chn in range(2):
                  c0 = chn * 384
                  pot = asmps.tile([128, HEADS, NSZ], F32, tag="scr")
                  po = pot[:].rearrange("p a b -> p (a b)")
                  for kb in range(2):
                      nc.tensor.matmul(po[:, 0:384],
                                       _r32(wo_sb[:, kb, hb * 128:(hb + 1) * 128]),
                                       _r32(oT_sb[:, kb, c0:c0 + 384]),
                                       start=(kb == 0), stop=(kb == 1))
                  enh = outp.tile([128, 384], F32, tag="enh")
                  nc.vector.scalar_tensor_tensor(
                      out=enh[:, :], in0=po[:, 0:384], scalar=bo_sb[:, hb:hb + 1],
                      in1=eq_sb[:, hb, c0:c0 + 384],
                      op0=mybir.AluOpType.add, op1=mybir.AluOpType.add)
                  nc.sync.dma_start(
                      out=oT_d.rearrange("(b p) m -> p b m", p=128)[:, hb, c0:c0 + 384],
                      in_=enh[:, :])


def _build_program():
    nc = bacc.Bacc("TRN2", target_bir_lowering=False, debug=False,
                   num_devices=NCORES)

    def din(name, shape, dt):
        return nc.dram_tensor(name, shape, dt, kind="ExternalInput").ap()

    ins = [
        din("e1T", [HID, N], F32R),
        din("e2T", [HID, N], F32R),
        din("eq1T", [HID, NQP], F32R),
        din("eq2T", [HID, NQP], F32R),
        din("wqT", [HID, HID], F32R),
        din("wkbT", [HID, D], F32R),
        din("wvT", [HID, HID], F32R),
        din("woT", [HID, HID], F32R),
        din("bq_h", [64, HEADS], F32),
        din("bkb", [64, 1], F32),
        din("bv2", [128, 2], F32),
        din("bo2", [128, 2], F32),
        din("a1T", [N, NQP], BF16),
        din("a2T", [N, NQP], BF16),
        din("id128", [128, 128], F32),
    ]
    outs = [
        nc.dram_tensor("o1T", [HID, NQP], F32, kind="ExternalOutput").ap(),
        nc.dram_tensor("o2T", [HID, NQP], F32, kind="ExternalOutput").ap(),
    ]
    with tile.TileContext(nc) as tc:
        with ExitStack() as ctx:
            _build_kernel(ctx, tc, ins, outs)
    nc.compile()
    return nc


_NC_CACHE = None
LAST_RESULTS = None


def kernel(kg1_emb, kg2_emb, alignment_matrix, Wq, bq, Wk, bk, Wv, bv, Wo, bo):
    global _NC_CACHE
    kg1 = np.asarray(kg1_emb, np.float32)
    kg2 = np.asarray(kg2_emb, np.float32)
    align = np.asarray(alignment_matrix, np.float32)
    Wq = np.asarray(Wq, np.float32); bq = np.asarray(bq, np.float32)
    Wk = np.asarray(Wk, np.float32); bk = np.asarray(bk, np.float32)
    Wv = np.asarray(Wv, np.float32); bv = np.asarray(bv, np.float32)
    Wo = np.asarray(Wo, np.float32); bo = np.asarray(bo, np.float32)

    # host-side layout prep (no reference math beyond weight folding of the
    # head-mean + scale, which is a constant-folding rewrite of the same graph)
    e1T = np.ascontiguousarray(kg1.T)
    e2T = np.ascontiguousarray(kg2.T)
    Wkb = Wk.reshape(HEADS, D, HID).mean(axis=0) * SCALE     # [64, 256]
    bkbv = (bk.reshape(HEADS, D).mean(axis=0) * SCALE).reshape(64, 1)
    wqT = np.ascontiguousarray(Wq.T)
    wkbT = np.ascontiguousarray(Wkb.T)
    wvT = np.ascontiguousarray(Wv.T)
    woT = np.ascontiguousarray(Wo.T)
    bq_h = np.ascontiguousarray(bq.reshape(HEADS, D).T)
    bv2 = np.ascontiguousarray(bv.reshape(2, 128).T)
    bo2 = np.ascontiguousarray(bo.reshape(2, 128).T)
    id128 = np.eye(128, dtype=np.float32)

    alignT_b = np.ascontiguousarray(align.T).astype(NPBF16)   # [m2, n1]
    align_b = align.astype(NPBF16)                            # [m1, n2]

    if _NC_CACHE is None:
        _NC_CACHE = _build_program()
    nc = _NC_CACHE

    in_maps = []
    for c in range(NCORES):
        r0 = c * NQ
        eq1 = np.zeros((HID, NQP), np.float32)
        eq1[:, 0:NQ] = e1T[:, r0:r0 + NQ]
        eq2 = np.zeros((HID, NQP), np.float32)
        eq2[:, 0:NQ] = e2T[:, r0:r0 + NQ]
        a1 = np.zeros((N, NQP), NPBF16)
        a1[:, 0:NQ] = alignT_b[:, r0:r0 + NQ]
        a2 = np.zeros((N, NQP), NPBF16)
        a2[:, 0:NQ] = align_b[:, r0:r0 + NQ]
        in_maps.append({
            "e1T": e1T, "e2T": e2T, "eq1T": eq1, "eq2T": eq2,
            "wqT": wqT, "wkbT": wkbT, "wvT": wvT, "woT": woT,
            "bq_h": bq_h, "bkb": bkbv, "bv2": bv2, "bo2": bo2,
            "a1T": a1, "a2T": a2, "id128": id128,
        })

    import os
    trace = os.environ.get("CKG_TRACE", "0") == "1"
    res = run_bass_kernel_spmd(nc, in_maps, core_ids=list(range(NCORES)),
                               trace=trace)
    global LAST_RESULTS
    LAST_RESULTS = res

    kg1_out = np.empty((N, HID), np.float32)
    kg2_out = np.empty((N, HID), np.float32)
    for c in range(NCORES):
        r0 = c * NQ
        kg1_out[r0:r0 + NQ, :] = res.results[c]["o1T"][:, 0:NQ].T
        kg2_out[r0:r0 + NQ, :] = res.results[c]["o2T"][:, 0:NQ].T
    return (kg1_out, kg2_out)



# revision 26
# speedup vs baseline: 1.3196x; 1.3196x over previous
"""Trainium2 Bass kernel for CrossKGAttention (bidirectional masked cross-attention
between two knowledge-graph embedding sets).

Math per direction (queries q_emb [Nq,256], kv kv_emb [Nk,256], mask A [Nq,Nk]):
  Q_i = q_emb @ Wq.T + bq            (head i slice, [Nq,64])
  Kbar = mean_i(kv_emb @ Wk.T + bk)  ([Nk,64])
  V_i  = kv_emb @ Wv.T + bv
  S_i  = Q_i @ Kbar.T * SCALE
  w    = softmax(S_i * A, axis=kv)
  out_i = w @ V_i ;  enhanced = q_emb + out @ Wo.T + bo

Device formulation: with E'' = exp(S) * A  (0 where A==0),
  numerator_i = E''_i^T V_i + sum_m V_m - A^T V_i
  denom_i     = (Nk - cnt) + sum_m E''_i          (cnt = mask column count, host)
since exp(S*A) = 1 + (exp(S)-1)*A = (1-A) + E''.  The E''^T V contraction runs
with E'' as the PE stationary so the output lands in natural [query, dim]
layout (no transposed assembly); A^T V runs as two extra matmuls per tile and
sum_m V enters as a rank-1 (-ones x vsum) seed of that accumulator.

exp is split between engines: most kv-chunk tiles use the ACT engine's exp
(scale=1/gamma folded in); every W_MOD-th tile instead uses a Schraudolph
bit-trick on DVE: gamma is pre-folded into the Kbar weights so one
scalar_tensor_tensor computes round(gamma*S + beta) * A -> int16, whose bits
reinterpreted as bf16 are exp(S)*A to ~4% (well inside tolerance; the masked
softmax is dominated by the 5940-entry uniform mass so E-term errors are
~1e-4 of the output).

Sharding: 8 cores; core c owns query rows [c*750,(c+1)*750) of both KGs.
K/V sources + weights replicated. Queries padded 750->768, kv padded
6000->6016 (47 full 128-chunks).
"""

import numpy as np
import ml_dtypes
from contextlib import ExitStack

import concourse.bass as bass
import concourse.tile as tile
from concourse import bacc, mybir
from concourse.bass_utils import run_bass_kernel_spmd

F32 = mybir.dt.float32
F32R = mybir.dt.float32r
BF16 = mybir.dt.bfloat16
I16 = mybir.dt.int16
NPBF16 = ml_dtypes.bfloat16

N = 6000
NKP = 6016              # padded kv entities (47 * 128)
HID = 256
HEADS = 4
D = 64
SCALE = D ** -0.5
NCORES = 8
NQ = N // NCORES        # 750 queries per core per direction
NQP = 768               # padded queries (3 chunks of 256)
NSZ = 256               # queries per n-chunk
NCHUNK = NQP // NSZ     # 3
MBS = 128               # kv-chunk size
NMB = NKP // MBS        # 47
MGRP = 4                # kv-chunks per mask DMA
NMG = 12                # 11 full groups of 4 + 1 tail of 3
# Schraudolph bf16 bit-trick constants: bits = round(GAM*S + BET); bits as
# bf16 ~= exp(S).  GAM folded into the Kbar projection weights on host.
GAM = 128.0 / np.log(2.0)
BET = 128.0 * 127.0 - 7.411
W_MOD = 4               # every W_MOD-th kv-chunk uses the DVE bit-trick exp
                        # (0 disables; tile is W-path iff mb % W_MOD == W_MOD-1)
FP8_SCORES = False      # score matmuls in fp8e4m3 + DoubleRow (2x PE rate);
                        # Kbar/Q packed [Ki=32, Ko=2, ...].  Off: the fp8
                        # repack doubles per-lane copy work on DVE (the
                        # pacer) while PE has slack.
F8 = mybir.dt.float8e4


def _is_w(mb):
    return W_MOD > 0 and (mb % W_MOD) == (W_MOD - 1)


def _build_kernel(ctx: ExitStack, tc, ins, outs):
    nc = tc.nc
    (e1T, e2T, eqb1, eqb2, eqf1, eqf2, wqT, wkbT, wvT, woT,
     bq_h, bkb, bkb8, bq8, bv2, bo2, a1T, a2T, dn01, dn02, es1, es2,
     id128) = ins
    o1T, o2T = outs

    ctx.enter_context(nc.allow_low_precision(reason="bf16/int16 attention core"))
    consts = ctx.enter_context(tc.tile_pool(name="consts", bufs=1))
    perdir = ctx.enter_context(tc.tile_pool(name="perdir", bufs=2))
    maskp = ctx.enter_context(tc.tile_pool(name="maskp", bufs=4))
    expp = ctx.enter_context(tc.tile_pool(name="expp", bufs=3))
    ep = ctx.enter_context(tc.tile_pool(name="ep", bufs=5))
    wp = ctx.enter_context(tc.tile_pool(name="wp", bufs=3))
    asmp = ctx.enter_context(tc.tile_pool(name="asmp", bufs=2))
    outp = ctx.enter_context(tc.tile_pool(name="outp", bufs=4))
    scrp = ctx.enter_context(tc.tile_pool(name="scrp", bufs=2, space="PSUM"))
    pvp = ctx.enter_context(tc.tile_pool(name="pvp", bufs=1, space="PSUM"))
    avp = ctx.enter_context(tc.tile_pool(name="avp", bufs=1, space="PSUM"))
    # dedicated 1-bank pool for W-tile scores: keeps the slow DVE stt from
    # holding a main scr buffer and stalling the ACT exp stream
    wscrp = ctx.enter_context(tc.tile_pool(name="wscrp", bufs=1, space="PSUM"))

    # ---- resident constants ----
    wq_sb = consts.tile([128, 2, HID], BF16)
    nc.sync.dma_start(out=wq_sb[:], in_=wqT.rearrange("(b p) h -> p b h", p=128))
    wkb_sb = consts.tile([128, 2, D], BF16)
    nc.sync.dma_start(out=wkb_sb[:], in_=wkbT.rearrange("(b p) d -> p b d", p=128))
    wv_sb = consts.tile([128, 2, HID], BF16)
    nc.sync.dma_start(out=wv_sb[:], in_=wvT.rearrange("(b p) h -> p b h", p=128))
    wo_sb = consts.tile([128, 2, HID], BF16)
    nc.sync.dma_start(out=wo_sb[:], in_=woT.rearrange("(b p) h -> p b h", p=128))
    id_sb = consts.tile([128, 128], BF16)
    nc.sync.dma_start(out=id_sb[:], in_=id128[:, :])
    bq_sb = consts.tile([64, HEADS], F32)
    nc.sync.dma_start(out=bq_sb[:], in_=bq_h[:, :])
    bkb_sb = consts.tile([64, 1], F32)
    nc.sync.dma_start(out=bkb_sb[:], in_=bkb[:, :])
    bkb8_sb = consts.tile([32, 2], F32)
    nc.sync.dma_start(out=bkb8_sb[:], in_=bkb8[:, :])
    bq8_sb = consts.tile([32, 2, HEADS], F32)
    nc.sync.dma_start(out=bq8_sb[:], in_=bq8[:, :, :])
    bo_sb = consts.tile([128, 2], F32)
    nc.sync.dma_start(out=bo_sb[:], in_=bo2[:, :])
    dn0_sb1 = consts.tile([128, 2 * NCHUNK], F32)
    nc.sync.dma_start(out=dn0_sb1[:], in_=dn01[:, :])
    dn0_sb2 = consts.tile([128, 2 * NCHUNK], F32)
    nc.sync.dma_start(out=dn0_sb2[:], in_=dn02[:, :])
    es_sb1 = consts.tile([128, 2], BF16)
    nc.sync.dma_start(out=es_sb1[:], in_=es1[:, :])
    es_sb2 = consts.tile([128, 2], BF16)
    nc.sync.dma_start(out=es_sb2[:], in_=es2[:, :])
    negones = consts.tile([1, 128], BF16)
    nc.vector.memset(negones[:], -1.0)

    def emit_proj(dirx):
        """Load per-direction inputs and run Kbar/Q/V projections."""
        st = {}
        st["maskT"] = a1T if dirx == 0 else a2T
        st["dn0"] = dn0_sb1 if dirx == 0 else dn0_sb2
        st["oT"] = o1T if dirx == 0 else o2T
        ekvT_d = e2T if dirx == 0 else e1T
        eqb_d = eqb1 if dirx == 0 else eqb2
        eqf_d = eqf1 if dirx == 0 else eqf2
        es_d = es_sb1 if dirx == 0 else es_sb2

        ekv_sb = perdir.tile([128, 2, NKP], BF16, tag="ekv")
        for i in range(4):
            s0 = i * (NKP // 4)
            nc.sync.dma_start(
                out=ekv_sb[:, :, s0:s0 + NKP // 4],
                in_=ekvT_d.rearrange("(b p) m -> p b m", p=128)
                [:, :, s0:s0 + NKP // 4])
        eqb_sb = perdir.tile([128, 2, NQP], BF16, tag="eqb")
        nc.sync.dma_start(out=eqb_sb[:],
                          in_=eqb_d.rearrange("(b p) m -> p b m", p=128))
        eqf_sb = perdir.tile([128, 2, NQP], F32, tag="eqf")
        nc.sync.dma_start(out=eqf_sb[:],
                          in_=eqf_d.rearrange("(b p) m -> p b m", p=128))
        st["eqf"] = eqf_sb

        if FP8_SCORES:
            kb_sb = perdir.tile([32, 2, NKP], F8, tag="kb")
            q_sb = perdir.tile([32, 2, HEADS, NQP], F8, tag="q")
        else:
            kb_sb = perdir.tile([64, NKP], BF16, tag="kb")
            q_sb = perdir.tile([64, HEADS, NQP], BF16, tag="q")
        vt_sb = perdir.tile([128, NMB, HEADS, D + 1], BF16, tag="vt")
        vsum_sb = perdir.tile([1, HEADS, D], BF16, tag="vsum")
        st["kb"] = kb_sb; st["q"] = q_sb
        st["vt"] = vt_sb; st["vsum"] = vsum_sb

        # Q projection first (depends only on the small eqb DMA)
        for h in range(HEADS):
            for chn in range(2):
                c0 = chn * 384
                ps = scrp.tile([128, HEADS, NSZ], F32, tag="scr")
                psv = ps[:].rearrange("p a b -> p (a b)")
                if FP8_SCORES:
                    for hf in range(2):
                        for kb in range(2):
                            nc.tensor.matmul(
                                psv[0:32, hf * 384:(hf + 1) * 384],
                                wq_sb[:, kb, h * D + hf * 32:h * D + hf * 32 + 32],
                                eqb_sb[:, kb, c0:c0 + 384],
                                start=(kb == 0), stop=(kb == 1))
                        nc.vector.tensor_scalar_add(
                            q_sb[:, hf, h, c0:c0 + 384],
                            psv[0:32, hf * 384:(hf + 1) * 384],
                            bq8_sb[:, hf, h:h + 1])
                else:
                    for kb in range(2):
                        nc.tensor.matmul(
                            psv[0:64, 0:384],
                            wq_sb[:, kb, h * D:(h + 1) * D],
                            eqb_sb[:, kb, c0:c0 + 384],
                            start=(kb == 0), stop=(kb == 1))
                    nc.vector.tensor_scalar_add(q_sb[:, h, c0:c0 + 384],
                                                psv[0:64, 0:384],
                                                bq_sb[:, h:h + 1])

        # Interleave Kbar chunks (DVE copies) with V chunks (ACT copies) so
        # both engines get work throughout the projection phase.
        nc.vector.memset(vt_sb[:, :, :, D:D + 1], 1.0)
        KCH = 512 if not FP8_SCORES else 376
        nkch = (NKP + KCH - 1) // KCH

        def emit_kbar_chunk(chn):
            c0 = chn * KCH
            cw = min(KCH, NKP - c0)
            ps = scrp.tile([128, HEADS, NSZ], F32, tag="scr")
            psv = ps[:].rearrange("p a b -> p (a b)")
            if FP8_SCORES:
                for hf in range(2):
                    for kb in range(2):
                        nc.tensor.matmul(psv[0:32, hf * cw:(hf + 1) * cw],
                                         wkb_sb[:, kb, hf * 32:(hf + 1) * 32],
                                         ekv_sb[:, kb, c0:c0 + cw],
                                         start=(kb == 0), stop=(kb == 1))
                    nc.vector.tensor_scalar_add(
                        kb_sb[:, hf, c0:c0 + cw],
                        psv[0:32, hf * cw:(hf + 1) * cw],
                        bkb8_sb[:, hf:hf + 1])
            else:
                for kb in range(2):
                    nc.tensor.matmul(psv[0:64, 0:cw],
                                     wkb_sb[:, kb, :],
                                     ekv_sb[:, kb, c0:c0 + cw],
                                     start=(kb == 0), stop=(kb == 1))
                nc.vector.tensor_scalar_add(kb_sb[:, c0:c0 + cw],
                                            psv[0:64, 0:cw], bkb_sb[:, 0:1])

        def emit_v_chunk(mb):
            m0 = mb * MBS
            ps = scrp.tile([128, HEADS, NSZ], F32, tag="scr")
            psv = ps[:].rearrange("p a b -> p (a b)")
            for kb in range(2):
                nc.tensor.matmul(psv[0:128, 0:HID],
                                 ekv_sb[:, kb, m0:m0 + MBS],
                                 wv_sb[:, kb, :],
                                 start=(kb == 0), stop=(kb == 1))
            src = psv[0:128, 0:HID].rearrange("p (h d) -> p h d", h=HEADS)
            nc.scalar.activation(out=vt_sb[0:128, mb, :, 0:D], in_=src,
                                 func=mybir.ActivationFunctionType.Copy)

        vmb = 0
        for chn in range(nkch):
            emit_kbar_chunk(chn)
            for _ in range(3):
                if vmb < NMB:
                    emit_v_chunk(vmb)
                    vmb += 1
        while vmb < NMB:
            emit_v_chunk(vmb)
            vmb += 1

        # vsum = sum_m V = (sum_m ekv_m) @ Wv^T, from host embedding sums
        psq = scrp.tile([128, HEADS, NSZ], F32, tag="scr")
        psqv = psq[:].rearrange("p a b -> p (a b)")
        for kb in range(2):
            nc.tensor.matmul(psqv[0:1, 0:HID],
                             es_d[:, kb:kb + 1],
                             wv_sb[:, kb, :],
                             start=(kb == 0), stop=(kb == 1))
        nc.vector.tensor_copy(vsum_sb[:].rearrange("p h d -> p (h d)"),
                              psqv[0:1, 0:HID])
        return st

    def emit_nt(st, nt):
        n0 = nt * NSZ
        maskT_d = st["maskT"]; dn0_d = st["dn0"]; oT_d = st["oT"]
        kb_sb = st["kb"]; q_sb = st["q"]
        vt_sb = st["vt"]; vsum_sb = st["vsum"]; eqf_sb = st["eqf"]

        pv = []
        avm = avp.tile([128, 2, HEADS, D], F32, tag="avb")
        avb = [avm[:, 0, :, :], avm[:, 1, :, :]]
        for qh in range(2):
            pv_t = pvp.tile([128, HEADS, D + 1], F32, tag=f"pv{qh}")
            pv.append(pv_t)
            # seed avb with -vsum broadcast (rank-1), then += A^T V
            nc.tensor.matmul(avb[qh], negones[:, :],
                             vsum_sb[:, :, :], start=True, stop=False)

        for mg in range(NMG):
            g0 = mg * MGRP
            gw = min(MGRP, NMB - g0)
            a_t = maskp.tile([128, MGRP, NSZ], BF16, tag="mask")
            nc.sync.dma_start(
                out=a_t[0:128, 0:gw, :],
                in_=maskT_d.rearrange("(mb p) n -> p mb n", p=128)
                [:, g0:g0 + gw, n0:n0 + NSZ])
            for g in range(gw):
                mb = g0 + g
                m0 = mb * MBS
                a_ap = a_t[0:128, g, :]
                if _is_w(mb):
                    # W-tile: scores into the dedicated 1-bank pool, masked
                    # Schraudolph exp entirely on DVE (half-tile granularity)
                    w16 = wp.tile([128, HEADS, NSZ], I16, tag="w16")
                    a_brd2 = bass.AP(a_ap.tensor, a_ap.offset,
                                     [a_ap.ap[0], [0, 2], a_ap.ap[1]])
                    for hp in range(2):
                        wscr = wscrp.tile([128, 2, NSZ], F32, tag="wscr")
                        wscrv = wscr[:].rearrange("p a b -> p (a b)")
                        if FP8_SCORES:
                            for hh in range(2):
                                nc.tensor.matmul(
                                    wscr[:, hh, :],
                                    kb_sb[:, :, m0:m0 + MBS],
                                    q_sb[:, :, 2 * hp + hh, n0:n0 + NSZ],
                                    start=True, stop=True,
                                    perf_mode=mybir.MatmulPerfMode.DoubleRow)
                        else:
                            nc.tensor.matmul(
                                wscrv[0:128, 0:512],
                                kb_sb[:, m0:m0 + MBS],
                                q_sb[:, 2 * hp:2 * hp + 2, n0:n0 + NSZ],
                                start=True, stop=True)
                        nc.vector.scalar_tensor_tensor(
                            out=w16[:, 2 * hp:2 * hp + 2, :],
                            in0=wscr[:, :, :], scalar=BET, in1=a_brd2,
                            op0=mybir.AluOpType.add, op1=mybir.AluOpType.mult)
                    e_src = w16[:].bitcast(BF16)
                else:
                    scr = scrp.tile([128, HEADS, NSZ], F32, tag="scr")
                    scrv = scr[:].rearrange("p a b -> p (a b)")
                    if FP8_SCORES:
                        for h in range(HEADS):
                            nc.tensor.matmul(
                                scr[:, h, :],
                                kb_sb[:, :, m0:m0 + MBS],
                                q_sb[:, :, h, n0:n0 + NSZ],
                                start=True, stop=True,
                                perf_mode=mybir.MatmulPerfMode.DoubleRow)
                    else:
                        for hp in range(2):
                            nc.tensor.matmul(
                                scrv[0:128, hp * 512:(hp + 1) * 512],
                                kb_sb[:, m0:m0 + MBS],
                                q_sb[:, 2 * hp:2 * hp + 2, n0:n0 + NSZ],
                                start=True, stop=True)
                    a_brd = bass.AP(a_ap.tensor, a_ap.offset,
                                    [a_ap.ap[0], [0, HEADS], a_ap.ap[1]])
                    exp_t = expp.tile([128, HEADS, NSZ], BF16, tag="exp")
                    nc.scalar.activation(out=exp_t[:, :, :],
                                         in_=scr[:, :, :],
                                         func=mybir.ActivationFunctionType.Exp,
                                         scale=1.0 / GAM)
                    e_t = ep.tile([128, HEADS, NSZ], BF16, tag="e")
                    nc.vector.tensor_tensor(out=e_t[:, :, :],
                                            in0=exp_t[:, :, :], in1=a_brd,
                                            op=mybir.AluOpType.mult)
                    e_src = e_t[:]
                for qh in range(2):
                    q0 = qh * 128
                    for h in range(HEADS):
                        nc.tensor.matmul(pv[qh][:, h, :],
                                         e_src[:, h, q0:q0 + 128],
                                         vt_sb[:, mb, h, :],
                                         start=(mb == 0),
                                         stop=(mb == NMB - 1))
                    nc.tensor.matmul(avb[qh][:, :, :],
                                     a_t[0:128, g, q0:q0 + 128],
                                     vt_sb[:, mb, :, 0:D],
                                     start=False, stop=(mb == NMB - 1))

        # ---- tail: normalize, DMA-transpose, Wo, residual ----
        wo_in = asmp.tile([128, 2, NSZ], BF16, tag="woin")
        for qh in range(2):
            den = outp.tile([128, HEADS, 1], F32, tag="den")
            nc.vector.tensor_scalar_add(den[:, :, :], pv[qh][:, :, D:D + 1],
                                        dn0_d[:, 2 * nt + qh:2 * nt + qh + 1])
            rec = outp.tile([128, HEADS, 1], F32, tag="rec")
            nc.vector.reciprocal(rec[:, :, :], den[:, :, :])
            avs = outp.tile([128, HEADS, D], BF16, tag="avs")
            nc.vector.tensor_copy(avs[:, :, :], avb[qh][:, :, :])
            t1 = outp.tile([128, HEADS, D], BF16, tag="t1")
            nc.vector.tensor_tensor(out=t1[:, :, :],
                                    in0=pv[qh][:, :, 0:D],
                                    in1=avs[:, :, :],
                                    op=mybir.AluOpType.subtract)
            u = outp.tile([128, HEADS, D], BF16, tag="u")
            for h in range(HEADS):
                nc.vector.tensor_scalar_mul(u[:, h, :], t1[:, h, :],
                                            rec[:, h, 0:1])
            uv = u[:].rearrange("p h d -> p (h d)")
            # [q,(h d)] -> [(h d), q] via xbar DMA transpose (bv folded into
            # bo on host: bo' = bo + Wo @ bv)
            nc.sync.dma_start_transpose(
                out=wo_in[:, :, qh * 128:qh * 128 + 128], in_=uv)
        # Wo PSUM reuses the avb bank (freed by the t1 reads above)
        wot = avp.tile([128, 2, HEADS, D], F32, tag="avb")
        for jh in range(2):
            wtv = wot[:, jh, :, :].rearrange("p h d -> p (h d)")
            for kb in range(2):
                nc.tensor.matmul(wtv[:, :],
                                 wo_sb[:, kb, jh * 128:(jh + 1) * 128],
                                 wo_in[:, kb, :],
                                 start=(kb == 0), stop=(kb == 1))
            enh = outp.tile([128, NSZ], F32, tag="enh")
            nc.vector.scalar_tensor_tensor(
                out=enh[:, :], in0=wtv[:, :],
                scalar=bo_sb[:, jh:jh + 1],
                in1=eqf_sb[:, jh, n0:n0 + NSZ],
                op0=mybir.AluOpType.add, op1=mybir.AluOpType.add)
            nc.sync.dma_start(
                out=oT_d.rearrange("(b p) m -> p b m", p=128)
                [:, jh, n0:n0 + NSZ],
                in_=enh[:, :])

    # Interleaved emission: dir1's projections are emitted between dir0's
    # query chunks so they fill engine bubbles during dir0's main loop.
    st0 = emit_proj(0)
    emit_nt(st0, 0)
    emit_nt(st0, 1)
    st1 = emit_proj(1)
    emit_nt(st0, 2)
    emit_nt(st1, 0)
    emit_nt(st1, 1)
    emit_nt(st1, 2)


def _build_program():
    nc = bacc.Bacc("TRN2", target_bir_lowering=False, debug=False,
                   num_devices=NCORES)

    def din(name, shape, dt):
        return nc.dram_tensor(name, shape, dt, kind="ExternalInput").ap()

    ins = [
        din("e1T", [HID, NKP], BF16),
        din("e2T", [HID, NKP], BF16),
        din("eqb1", [HID, NQP], BF16),
        din("eqb2", [HID, NQP], BF16),
        din("eqf1", [HID, NQP], F32),
        din("eqf2", [HID, NQP], F32),
        din("wqT", [HID, HID], BF16),
        din("wkbT", [HID, D], BF16),
        din("wvT", [HID, HID], BF16),
        din("woT", [HID, HID], BF16),
        din("bq_h", [64, HEADS], F32),
        din("bkb", [64, 1], F32),
        din("bkb8", [32, 2], F32),
        din("bq8", [32, 2, HEADS], F32),
        din("bv2", [128, 2], F32),
        din("bo2", [128, 2], F32),
        din("a1T", [NKP, NQP], BF16),
        din("a2T", [NKP, NQP], BF16),
        din("dn01", [128, 2 * NCHUNK], F32),
        din("dn02", [128, 2 * NCHUNK], F32),
        din("es1", [128, 2], BF16),
        din("es2", [128, 2], BF16),
        din("id128", [128, 128], BF16),
    ]
    outs = [
        nc.dram_tensor("o1T", [HID, NQP], F32, kind="ExternalOutput").ap(),
        nc.dram_tensor("o2T", [HID, NQP], F32, kind="ExternalOutput").ap(),
    ]
    with tile.TileContext(nc) as tc:
        with ExitStack() as ctx:
            _build_kernel(ctx, tc, ins, outs)
    nc.compile()
    return nc


_NC_CACHE = None
LAST_RESULTS = None


def kernel(kg1_emb, kg2_emb, alignment_matrix, Wq, bq, Wk, bk, Wv, bv, Wo, bo):
    global _NC_CACHE
    kg1 = np.asarray(kg1_emb, np.float32)
    kg2 = np.asarray(kg2_emb, np.float32)
    align = np.asarray(alignment_matrix, np.float32)
    Wq = np.asarray(Wq, np.float32); bq = np.asarray(bq, np.float32)
    Wk = np.asarray(Wk, np.float32); bk = np.asarray(bk, np.float32)
    Wv = np.asarray(Wv, np.float32); bv = np.asarray(bv, np.float32)
    Wo = np.asarray(Wo, np.float32); bo = np.asarray(bo, np.float32)

    # host-side layout prep: pads, transposes, dtype casts, weight folding
    # (head-mean + softmax scale + Schraudolph gamma are constant rewrites),
    # and linear input summaries (mask column counts, embedding sums).
    e1p = np.zeros((NKP, HID), np.float32); e1p[0:N] = kg1
    e2p = np.zeros((NKP, HID), np.float32); e2p[0:N] = kg2
    e1T = np.ascontiguousarray(e1p.T).astype(NPBF16)
    e2T = np.ascontiguousarray(e2p.T).astype(NPBF16)
    Wkb = Wk.reshape(HEADS, D, HID).mean(axis=0) * (SCALE * GAM)
    bkbv = (bk.reshape(HEADS, D).mean(axis=0) * (SCALE * GAM)).reshape(64, 1)
    wqT = np.ascontiguousarray(Wq.T).astype(NPBF16)
    wkbT = np.ascontiguousarray(Wkb.T).astype(NPBF16)
    wvT = np.ascontiguousarray(Wv.T).astype(NPBF16)
    woT = np.ascontiguousarray(Wo.T).astype(NPBF16)
    bq_h = np.ascontiguousarray(bq.reshape(HEADS, D).T.astype(np.float32))
    bkb8 = np.ascontiguousarray(bkbv.reshape(2, 32).T.astype(np.float32))
    bq8 = np.ascontiguousarray(
        bq.reshape(HEADS, 2, 32).transpose(2, 1, 0).astype(np.float32))
    bv2 = np.ascontiguousarray(bv.reshape(2, 128).T.astype(np.float32))
    bo_folded = bo + Wo @ bv          # bv applied pre-Wo == Wo@bv post-Wo
    bo2 = np.ascontiguousarray(bo_folded.reshape(2, 128).T.astype(np.float32))
    id128 = np.eye(128, dtype=NPBF16)
    es1v = np.ascontiguousarray(  # kv sums for dir1 (kv = kg2)
        kg2.sum(axis=0).reshape(2, 128).T.astype(NPBF16))
    es0v = np.ascontiguousarray(  # kv sums for dir0? naming: dir0 kv = kg2
        kg1.sum(axis=0).reshape(2, 128).T.astype(NPBF16))
    cnt1 = align.sum(axis=1)          # per kg1 query: count of kv=kg2 nbrs
    cnt2 = align.sum(axis=0)          # per kg2 query: count of kv=kg1 nbrs

    a1full = np.zeros((NKP, N), NPBF16)
    a1full[0:N] = np.ascontiguousarray(align.T).astype(NPBF16)  # [m2, n1]
    a2full = np.zeros((NKP, N), NPBF16)
    a2full[0:N] = align.astype(NPBF16)                           # [m1, n2]

    if _NC_CACHE is None:
        _NC_CACHE = _build_program()
    nc = _NC_CACHE

    in_maps = []
    for c in range(NCORES):
        r0 = c * NQ
        eqb1 = np.zeros((HID, NQP), NPBF16)
        eqf1 = np.zeros((HID, NQP), np.float32)
        eqf1[:, 0:NQ] = kg1.T[:, r0:r0 + NQ]
        eqb1[:, 0:NQ] = eqf1[:, 0:NQ].astype(NPBF16)
        eqb2 = np.zeros((HID, NQP), NPBF16)
        eqf2 = np.zeros((HID, NQP), np.float32)
        eqf2[:, 0:NQ] = kg2.T[:, r0:r0 + NQ]
        eqb2[:, 0:NQ] = eqf2[:, 0:NQ].astype(NPBF16)
        a1 = np.zeros((NKP, NQP), NPBF16)
        a1[:, 0:NQ] = a1full[:, r0:r0 + NQ]
        a2 = np.zeros((NKP, NQP), NPBF16)
        a2[:, 0:NQ] = a2full[:, r0:r0 + NQ]
        dn01 = np.full((NQP,), float(N), np.float32)
        dn01[0:NQ] -= cnt1[r0:r0 + NQ]
        dn02 = np.full((NQP,), float(N), np.float32)
        dn02[0:NQ] -= cnt2[r0:r0 + NQ]
        # [q] -> [128, 6]: q = nt*256 + qh*128 + p  -> col = nt*2+qh
        dn01 = np.ascontiguousarray(dn01.reshape(6, 128).T)
        dn02 = np.ascontiguousarray(dn02.reshape(6, 128).T)
        in_maps.append({
            "e1T": e1T, "e2T": e2T,
            "eqb1": eqb1, "eqb2": eqb2, "eqf1": eqf1, "eqf2": eqf2,
            "wqT": wqT, "wkbT": wkbT, "wvT": wvT, "woT": woT,
            "bq_h": bq_h, "bkb": bkbv, "bkb8": bkb8, "bq8": bq8,
            "bv2": bv2, "bo2": bo2,
            "a1T": a1, "a2T": a2, "dn01": dn01, "dn02": dn02,
            "es1": es1v, "es2": es0v, "id128": id128,
        })

    import os
    trace = os.environ.get("CKG_TRACE", "0") == "1"
    res = run_bass_kernel_spmd(nc, in_maps, core_ids=list(range(NCORES)),
                               trace=trace)
    global LAST_RESULTS
    LAST_RESULTS = res

    kg1_out = np.empty((N, HID), np.float32)
    kg2_out = np.empty((N, HID), np.float32)
    for c in range(NCORES):
        r0 = c * NQ
        kg1_out[r0:r0 + NQ, :] = res.results[c]["o1T"][:, 0:NQ].T
        kg2_out[r0:r0 + NQ, :] = res.results[c]["o2T"][:, 0:NQ].T
    return (kg1_out, kg2_out)
